# revision 38
# baseline (speedup 1.0000x reference)
"""Trainium2 Bass kernel for nn_Attention_86217173500445.

Cross-attention block: shared QKV projections over two inputs (base/target),
4 attention streams (bb, tt, bt, tb), shared output projection.

Strategy: data-parallel over batch (B=32 -> 4 per core on 8 cores), weights
replicated, zero collectives.  Per-core compute is a fully-fused bf16
pipeline (1 column/cycle on the PE, fp32 PSUM accumulation; rel err ~7e-3
vs the 2e-2 gate):

  - x and the 4 weight matrices are HOST-cast to bf16 and uploaded as bf16
    (the kernel consumed them in bf16 anyway) -- halves the input DMA and
    removes all on-chip staging casts.
  - x is transposed on-chip at the bf16 1-cycle/row rate into XT [C, S].
  - Q/K projections produce transposed outputs QT/KT [C, S] directly
    (bias applied by the ACT psum drain); V projection produces
    natural-layout V [S, C] (bias added by the DVE drain).
  - Scores are computed transposed (scoresT[k, q]) so the ACT-engine exp
    output feeds the AV matmul as the moving operand with no transposes.
    Max-subtraction is skipped (scores ~ N(0,1), exp is safe).
  - V carries two trailing all-ones columns, so each AV matmul lands the
    head's softmax row-sum in psum rows 64/65 for free (the dedicated
    row-sum matmuls of the earlier version were ~60us of PE time).
  - Normalization: the two rowsum rows are copied (ACT j1 / DVE j0) to a
    [66, ...] SBUF tile, broadcast along channels by a 2-row stationary
    matmul at tile position (64, 0), reciprocal'd on DVE *after* the
    broadcast (reciprocal_approx_fast is FD-bound, so 128 rows cost the
    same as 2 -- and the custom uop misreads psum at a nonzero base
    partition on HW, so it must run at base 0), and applied by a GpSimd
    multiply (DVE multiply for the last batch, where GpSimd serialization
    would stretch the epilogue).
  - Output projection consumes the normalized attention output as the
    stationary operand, producing natural [S, C] tiles DMA'd to DRAM in
    per-512/256-channel chunks alternating between two DMA queues.

Scheduling: engines execute their queues strictly in order, so the static
emission order IS the schedule.  The PE clock is HAM-gated: it runs at
1.2 GHz unless the PE stays busy through consecutive 3.4us activity
windows, so every pair slot must carry enough independent PE work:
  - a dense dummy-matmul burst under the (DMA-bound) weight-load prologue
    warms the PE clock before real work,
  - per pair slot: scores(sti0) / AV(j0, pair-3) / scores(sti1) /
    AV(j1, pair-3) / rowsum-broadcast(pair-5) are interleaved so the PE
    never catches up with the ACT exp chain,
  - PE fillers: prev batch's sigma-1 outproj at slots 0-2/4, next batch's
    transposes at 3/5, V projection at 6-7, Q/K projection at 8-9 (spilling
    into the tail), this batch's sigma-0 outproj at 10-11; slots with no
    real fill get dummy-matmul padding so the HAM window never reads idle.
Engine balance: exp + QK-bias drains + rowsum j1 copies on ACT; OT drains
(copy + partition-shift shuffle), rowsum j0 copies, reciprocals and V/out
biases on DVE; normalize multiplies and out-DMA issue on GpSimd.
"""

import numpy as np

import concourse.bass as bass
import concourse.bacc as bacc
import concourse.mybir as mybir
import concourse.tile as tile
from concourse.bass_utils import run_bass_kernel_spmd
from concourse.masks import make_identity

FP32 = mybir.dt.float32
BF16 = mybir.dt.bfloat16
AF = mybir.ActivationFunctionType

H, DH, S, C = 12, 64, 197, 768
NCO = C // 128  # 6 channel chunks
SCALE = DH ** -0.5
S_TILES = [(0, 128), (128, 69)]
N_CHUNKS = [(0, 512), (512, 256)]
# (key/value source, query source) -> output stream index; 0=base, 1=target
STREAM_IDX = {(0, 0): 0, (0, 1): 3, (1, 1): 1, (1, 0): 2}
N_CORES = 8
S2 = 2 * S  # query axis covers both query sources side by side
DV = DH + 2  # V head stride: 64 data columns + 2 all-ones (rowsum) columns
DEBUG_DUMPS = False


def build_nc(B_L):
    nc = bacc.Bacc("TRN2", target_bir_lowering=False, debug=False,
                   num_devices=N_CORES)

    x_in = {
        0: nc.dram_tensor("x_base", [B_L, S, C], BF16, kind="ExternalInput"),
        1: nc.dram_tensor("x_target", [B_L, S, C], BF16, kind="ExternalInput"),
    }
    w_dram, b_dram = {}, {}
    for nm in ("q", "k", "v", "p"):
        w_dram[nm] = nc.dram_tensor(f"W{nm}", [C, C], BF16, kind="ExternalInput")
        b_dram[nm] = nc.dram_tensor(f"b{nm}", [C], FP32, kind="ExternalInput")
    out_d = nc.dram_tensor("out", [4, B_L, S, C], FP32, kind="ExternalOutput")
    dbg = {}
    if DEBUG_DUMPS:
        dbg["XT"] = nc.dram_tensor("dbg_XT", [128, NCO, 2, S], BF16,
                                   kind="ExternalOutput")
        dbg["QT"] = nc.dram_tensor("dbg_QT", [128, NCO, 2, S], BF16,
                                   kind="ExternalOutput")
        dbg["KT"] = nc.dram_tensor("dbg_KT", [128, NCO, 2, S], BF16,
                                   kind="ExternalOutput")
        dbg["V"] = nc.dram_tensor("dbg_V", [69, 2, 2, H, DV], BF16,
                                  kind="ExternalOutput")
        dbg["rr0"] = nc.dram_tensor("dbg_rr0", [2, NCO, S2], BF16,
                                    kind="ExternalOutput")
        dbg["OTraw0"] = nc.dram_tensor("dbg_OTraw0", [128, NCO, S2], FP32,
                                       kind="ExternalOutput")
        dbg["OTn0"] = nc.dram_tensor("dbg_OTn0", [128, NCO, S2], BF16,
                                     kind="ExternalOutput")

    with tile.TileContext(nc) as tc:
        with (
            tc.tile_pool(name="const", bufs=1) as constp,
            tc.tile_pool(name="stage", bufs=4) as stagep,
            tc.tile_pool(name="wsb", bufs=1) as wp,
            tc.tile_pool(name="xt", bufs=2) as xtp,
            tc.tile_pool(name="qkv", bufs=2) as qkvp,
            tc.tile_pool(name="expp", bufs=20) as expp,
            tc.tile_pool(name="ot", bufs=2) as otp,
            tc.tile_pool(name="rpool", bufs=2) as rp,
            tc.tile_pool(name="y2", bufs=4) as y2p,
            tc.tile_pool(name="ps_sc", bufs=3, space="PSUM") as ps_sc,
            tc.tile_pool(name="ps_av", bufs=2, space="PSUM") as ps_av,
            tc.tile_pool(name="ps_sh", bufs=2, space="PSUM") as ps_sh,
            tc.tile_pool(name="ps_pr", bufs=1, space="PSUM") as ps_pr,
        ):
            # ---- constants ----
            ident = constp.tile([128, 128], BF16)
            make_identity(nc, ident)

            # E2[64, c] = 1 iff c < 64; E2[65, c] = 1 iff c >= 64.  The
            # 2-row stationary that broadcasts the per-head (j0, j1)
            # 1/rowsum rows across their 64-channel groups.
            E2 = constp.tile([66, 128], BF16, name="E2")
            nc.gpsimd.memset(E2, 1.0)
            nc.gpsimd.affine_select(
                out=E2[64:66, :], in_=E2[64:66, :],
                compare_op=mybir.AluOpType.is_ge, fill=0.0,
                base=0, pattern=[[1, 128]], channel_multiplier=-DH)
            nc.gpsimd.affine_select(
                out=E2[64:66, :], in_=E2[64:66, :],
                compare_op=mybir.AluOpType.is_ge, fill=0.0,
                base=DH - 1, pattern=[[-1, 128]], channel_multiplier=DH)

            # per-partition channel biases for the transposed Q/K outputs
            bqk_sb = {}
            for nm in ("q", "k"):
                t = constp.tile([128, NCO], FP32, name=f"b{nm}_sb")
                nc.gpsimd.dma_start(
                    out=t, in_=b_dram[nm].rearrange("(ko p) -> p ko", p=128))
                bqk_sb[nm] = t
            # V / out-proj biases broadcast along partitions (DVE add)
            bbc = {}
            for nm in ("v", "p"):
                t = constp.tile([128, C], FP32, name=f"b{nm}_bc")
                src_ap = b_dram[nm][:]
                bcast = bass.AP(tensor=src_ap.tensor, offset=src_ap.offset,
                                ap=[[0, 128]] + list(src_ap.ap))
                nc.gpsimd.dma_start(out=t, in_=bcast)
                bbc[nm] = t
            bbc_v, bbc_p = bbc["v"], bbc["p"]

            # ---- PE warm-up: dense dummy matmuls under the weight-load
            # prologue so HAM un-throttles the PE clock before real work ----
            warm_w = constp.tile([128, 512], BF16, name="warm_w")
            nc.vector.memset(warm_w, 0.125)

            def emit_warm(n):
                for _ in range(n):
                    wp_ = ps_sc.tile([128, 512], FP32, tag="sc", name="warm_ps")
                    nc.tensor.matmul(wp_[:, :512], lhsT=warm_w[:, :128],
                                     rhs=warm_w[:, :512], start=True, stop=True)

            emit_warm(36)

            # ---- prefetch batch-0 x tiles ahead of the weight loads ----
            x_tiles = {}

            def emit_x_dma(b):
                for src in (0, 1):
                    for (s0, s_sz) in S_TILES:
                        xb = stagep.tile([128, C], BF16, tag="xb", name="xb")
                        nc.sync.dma_start(out=xb[:s_sz, :],
                                          in_=x_in[src][b, s0:s0 + s_sz, :])
                        x_tiles[(b, src, s0)] = xb

            emit_x_dma(0)

            # ---- weights: DMA fp32 then DVE-cast to bf16 ----
            W_sb = {}

            def emit_w_load(nm):
                W_sb[nm] = wp.tile([128, NCO, C], BF16, tag=f"w{nm}",
                                   name=f"W{nm}_sb")
                for ko in range(NCO):
                    nc.sync.dma_start(out=W_sb[nm][:, ko, :],
                                      in_=w_dram[nm][ko * 128:(ko + 1) * 128, :])

            for nm in ("v", "q", "k"):
                emit_w_load(nm)

            # ---- per-batch persistent tiles, (re)allocated each iteration ----
            state = {}

            def emit_transpose_piece(b, src, sti, use_act):
                """Transpose one (src, s-tile) slab of x into XT: 6 channel
                chunks as two psum-bank groups, each drained by one bulk
                copy so the phase stays PE-dense instead of copy-paced."""
                s0, s_sz = S_TILES[sti]
                xb = x_tiles[(b, src, s0)]
                XT = state[("XT", b)]
                for g, (c0, ncg) in enumerate(((0, 4), (4, 2))):
                    pt = ps_sh.tile([128, 4, 128], BF16, tag="sh",
                                    name="pt")
                    for ci in range(ncg):
                        co = c0 + ci
                        nc.tensor.transpose(
                            pt[:, ci, :s_sz],
                            xb[:s_sz, co * 128:(co + 1) * 128],
                            ident[:s_sz, :s_sz])
                    dst = XT[:, c0:c0 + ncg, src, s0:s0 + s_sz]
                    if use_act and (src + g) % 2 == 0:
                        nc.scalar.copy(out=dst, in_=pt[:, :ncg, :s_sz])
                    else:
                        nc.vector.tensor_copy(out=dst, in_=pt[:, :ncg, :s_sz])

            def emit_transposes(b, pieces=None, use_act=True):
                if ("XT", b) not in state:
                    state[("XT", b)] = xtp.tile([128, NCO, 2, S], BF16,
                                                tag="xt", name="XT")
                if pieces is None:
                    pieces = [(src, sti) for src in (0, 1) for sti in (0, 1)]
                for src, sti in pieces:
                    emit_transpose_piece(b, src, sti, use_act=use_act)

            def _emit_qk_one(nm, OUT, m, b):  # noqa: unused b kept
                XT = state[("XT", b)]
                pp = ps_sh.tile([128, 2, S], FP32, tag="sh", name="pp")
                for k in range(NCO):
                    nc.tensor.matmul(
                        pp[:], lhsT=W_sb[nm][:, k, m * 128:(m + 1) * 128],
                        rhs=XT[:, k, :, :],
                        start=(k == 0), stop=(k == NCO - 1))
                nc.scalar.activation(
                    out=OUT[:, m, :, :], in_=pp[:], func=AF.Identity,
                    bias=bqk_sb[nm][:, m:m + 1], scale=1.0)

            def emit_qk_half(b, half):
                """Q/K projection chunks m in [3*half, 3*half+3)."""
                if half == 0:
                    state[("QT", b)] = qkvp.tile([128, NCO, 2, S], BF16,
                                                 tag="qt", name="QT")
                    state[("KT", b)] = qkvp.tile([128, NCO, 2, S], BF16,
                                                 tag="kt", name="KT")
                for m in range(3 * half, 3 * half + 3):
                    _emit_qk_one("q", state[("QT", b)], m, b)
                for m in range(3 * half, 3 * half + 3):
                    _emit_qk_one("k", state[("KT", b)], m, b)

            def emit_vproj_half(b, src):
                """V projection for one source; bias rides the matmul as a
                ones-row accumulation, so the drain is a plain DVE copy."""
                XT = state[("XT", b)]
                if src == 0:
                    V_sb = qkvp.tile([128, 2, 2, H, DV], BF16, tag="v",
                                     name="V_sb")
                    state[("V", b)] = V_sb
                    # the two all-ones rowsum columns per head
                    nc.gpsimd.memset(V_sb[:, :, :, :, DH:DV], 1.0)
                V_sb = state[("V", b)]
                for sti, (s0, s_sz) in enumerate(S_TILES):
                    for (n0, n_sz) in N_CHUNKS:
                        pv = ps_sh.tile([128, 512], FP32, tag="sh",
                                        name="pv")
                        for k in range(NCO):
                            nc.tensor.matmul(
                                pv[:s_sz, :n_sz],
                                lhsT=XT[:, k, src, s0:s0 + s_sz],
                                rhs=W_sb["v"][:, k, n0:n0 + n_sz],
                                start=(k == 0), stop=(k == NCO - 1))
                        nh, h0 = n_sz // DH, n0 // DH
                        nc.vector.tensor_add(
                            out=V_sb[:s_sz, src, sti, h0:h0 + nh, :DH],
                            in0=pv[:s_sz, :n_sz].rearrange(
                                "p (h d) -> p h d", d=DH),
                            in1=bbc_v[:s_sz, n0:n0 + n_sz].rearrange(
                                "p (h d) -> p h d", d=DH))

            def emit_proj(b):
                emit_transposes(b, use_act=False)
                emit_vproj_half(b, 0)
                emit_qk_half(b, 0)
                emit_vproj_half(b, 1)
                emit_qk_half(b, 1)

            def emit_scores_exp(b, sigma, hh, sti):
                """Scores + exp for head pair hh, one s-tile."""
                QT, KT = state[("QT", b)], state[("KT", b)]
                s0, s_sz = S_TILES[sti]
                et = state.setdefault(("e", sigma, hh), {})
                for j in (0, 1):
                    hp = j * DH
                    psc = ps_sc.tile([128, 512], FP32, tag="sc", name="psc")
                    nc.tensor.matmul(
                        psc[:s_sz, :S2],
                        lhsT=KT[hp:hp + DH, hh, sigma, s0:s0 + s_sz],
                        rhs=QT[hp:hp + DH, hh, :, :],
                        start=True, stop=True)
                    e = expp.tile([128, S2], BF16, tag="exp", name="e")
                    nc.scalar.activation(out=e[:s_sz, :],
                                         in_=psc[:s_sz, :S2],
                                         func=AF.Exp, scale=float(SCALE))
                    et[(sti, j)] = e

            def emit_av_mms(b, sigma, hh, j):
                """AV matmuls for one head of pair hh (rowsum rides rows
                64/65 via the ones columns of V)."""
                V_sb = state[("V", b)]
                et = state[("e", sigma, hh)]
                h = 2 * hh + j
                pav = ps_av.tile([128, 512], FP32, tag="av", name="pav")
                for sti, (s0, s_sz) in enumerate(S_TILES):
                    nc.tensor.matmul(
                        pav[:DV, :S2],
                        lhsT=V_sb[:s_sz, sigma, sti, h, :],
                        rhs=et[(sti, j)][:s_sz, :],
                        start=(sti == 0), stop=(sti == 1))
                state[("pav", sigma, hh, j)] = pav

            def emit_av_drains(sigma, hh):
                """Drain O rows to OT_raw, 1/rowsum to rr2b (bf16)."""
                OT_raw = state[("OT", sigma)]
                rr2b = state[("rr2b", sigma)]
                state.pop(("e", sigma, hh))
                pav0 = state.pop(("pav", sigma, hh, 0))
                pav1 = state.pop(("pav", sigma, hh, 1))
                nc.vector.tensor_copy(out=OT_raw[0:DH, hh, :],
                                       in_=pav0[0:DH, :S2])
                nc.vector.stream_shuffle(
                    out=OT_raw[DH:2 * DH, hh, :],
                    in_=pav1[0:DH, :S2], mask=list(range(32)))
                # rowsum rows stay at partitions 64/65 (bf16): j1's copy
                # fills both, j0's overwrites row 64
                nc.scalar.copy(out=rr2b[64:66, hh, :],
                               in_=pav1[64:66, :S2])
                nc.vector.tensor_copy(out=rr2b[64:65, hh, :],
                                      in_=pav0[64:65, :S2])

            def emit_normpair(b, sigma, hh):
                """Broadcast the rowsums along channels (PE), reciprocal of
                the broadcast (DVE, base 0), normalize multiply (GpSimd)."""
                OT_raw = state[("OT", sigma)]
                rr2b = state[("rr2b", sigma)]
                OT = state[("OTn", b, sigma)]
                pr = ps_pr.tile([128, 512], FP32, tag="pr", name="pr")
                nc.tensor.matmul(pr[:, :S2],
                                 lhsT=E2[64:66, :],
                                 rhs=rr2b[64:66, hh, :],
                                 start=True, stop=True)
                rbc = rp.tile([128, S2], FP32, tag="rbc", name="rbc", bufs=2)
                nc.vector.reciprocal_approx_fast(out=rbc, in_=pr[:, :S2])
                if b == B_L - 1 and (sigma, hh) >= (1, 2):
                    mul = nc.vector.tensor_mul if hh % 2 else \
                        nc.gpsimd.tensor_mul
                else:
                    mul = nc.gpsimd.tensor_mul
                mul(out=OT[:, hh, :], in0=OT_raw[:, hh, :], in1=rbc)

            def emit_outproj(b, sigma, qs, sti):
                """One [s_tile, C] slab of the output projection."""
                OT = state[("OTn", b, sigma)]
                stream = STREAM_IDX[(sigma, qs)]
                s0, s_sz = S_TILES[sti]
                y = y2p.tile([128, C], FP32, tag="y2")
                for (n0, n_sz) in N_CHUNKS:
                    py = ps_sh.tile([128, 512], FP32, tag="sh", name="py")
                    for k in range(NCO):
                        nc.tensor.matmul(
                            py[:s_sz, :n_sz],
                            lhsT=OT[:, k, qs * S + s0: qs * S + s0 + s_sz],
                            rhs=W_sb["p"][:, k, n0:n0 + n_sz],
                            start=(k == 0), stop=(k == NCO - 1))
                    nc.vector.tensor_add(
                        out=y[:s_sz, n0:n0 + n_sz],
                        in0=py[:s_sz, :n_sz],
                        in1=bbc_p[:s_sz, n0:n0 + n_sz])
                    dma_q = nc.sync if (qs + sti + (n0 > 0)) % 2 == 0 \
                        else nc.gpsimd
                    dma_q.dma_start(
                        out=out_d[stream, b, s0:s0 + s_sz, n0:n0 + n_sz],
                        in_=y[:s_sz, n0:n0 + n_sz])

            # ---- main loop: software-pipelined emission.  Tail work and
            # the next batch's projections are spread across the pair
            # slots so the PE always has independent fill work behind the
            # exp dependency chain. ----
            emit_proj(0)
            emit_w_load("p")
            for b in range(B_L):
                for sigma in (0, 1):
                    state[("OT", sigma)] = otp.tile(
                        [128, NCO, S2], FP32, tag="otraw", name="OT", bufs=2)
                    state[("rr2b", sigma)] = rp.tile(
                        [66, NCO, S2], BF16, tag="rr2b", name="rr2b", bufs=2)
                    state[("OTn", b, sigma)] = otp.tile(
                        [128, NCO, S2], BF16, tag="ot", name="OTn", bufs=3)
                pairs = [(sigma, hh) for sigma in (0, 1) for hh in range(NCO)]
                if DEBUG_DUMPS and b == 0:
                    dbg_qt, dbg_kt = state[("QT", 0)], state[("KT", 0)]
                if b + 1 < B_L:
                    emit_x_dma(b + 1)
                for idx, (sigma, hh) in enumerate(pairs):
                    # interleave scores with the AV matmuls of pair idx-3
                    # (3 slots of lead so the PE never catches up with the
                    # ACT exp chain) and the rowsum broadcast of pair idx-5
                    emit_scores_exp(b, sigma, hh, 0)
                    if idx > 2:
                        emit_av_mms(b, *pairs[idx - 3], 0)
                    emit_scores_exp(b, sigma, hh, 1)
                    if idx > 2:
                        emit_av_mms(b, *pairs[idx - 3], 1)
                        emit_av_drains(*pairs[idx - 3])
                    if idx > 4:
                        emit_normpair(b, *pairs[idx - 5])
                    # PE fill: every slot gets independent matmul work (an
                    # idle PE gets clocked down to 1.2 GHz): prev batch's
                    # sigma-1 outproj at 0-2/4, next batch's transposes at
                    # 3/5 and V projection at 6-7, Q/K projection at 8-9,
                    # this batch's first sigma-0 outproj at 10-11.  Slots
                    # with no fill available get dummy-matmul padding.
                    fill = False
                    if idx in (0, 1, 2, 4):
                        if b > 0:
                            sl = idx if idx < 3 else 3
                            emit_outproj(b - 1, 1, sl // 2, sl % 2)
                            fill = True
                    elif idx in (3, 5):
                        if b + 1 < B_L:
                            s = 0 if idx == 3 else 1
                            emit_transposes(b + 1, [(s, 0), (s, 1)])
                            fill = True
                    elif idx in (6, 7):
                        if b + 1 < B_L:
                            emit_vproj_half(b + 1, idx - 6)
                            fill = True
                    elif idx == 8:
                        if b + 1 < B_L:
                            emit_qk_half(b + 1, 0)
                            fill = True
                    elif idx == 9:
                        fill = b + 1 < B_L  # qk_half(0) spills into this slot
                    elif idx >= 10:
                        emit_outproj(b, 0, (idx - 10) // 2, (idx - 10) % 2)
                        fill = True
                    if not fill:
                        emit_warm(6 if b == 0 else 2)
                # drain the last three pairs, interleaved with the next
                # batch's remaining projections so the XT-drain -> QK
                # dependency and the exp -> AV chains never idle the PE
                emit_av_mms(b, *pairs[9], 0)
                emit_av_mms(b, *pairs[9], 1)
                emit_av_drains(*pairs[9])
                if b + 1 < B_L:
                    emit_qk_half(b + 1, 1)
                else:
                    emit_warm(2)
                emit_av_mms(b, *pairs[10], 0)
                emit_normpair(b, *pairs[7])
                emit_av_mms(b, *pairs[10], 1)
                emit_av_drains(*pairs[10])
                emit_normpair(b, *pairs[8])
                emit_outproj(b, 0, 1, 0)
                emit_av_mms(b, *pairs[11], 0)
                emit_normpair(b, *pairs[9])
                emit_av_mms(b, *pairs[11], 1)
                emit_av_drains(*pairs[11])
                emit_normpair(b, *pairs[10])
                emit_outproj(b, 0, 1, 1)
                emit_normpair(b, *pairs[11])
                if b + 1 >= B_L:
                    emit_warm(6)
                    emit_outproj(b, 1, 0, 0)
                    emit_warm(3)
                    emit_outproj(b, 1, 0, 1)
                    emit_warm(3)
                    emit_outproj(b, 1, 1, 0)
                    emit_outproj(b, 1, 1, 1)
                if DEBUG_DUMPS and b == 0:
                    nc.sync.dma_start(out=dbg["XT"][:], in_=state[("XT", 0)][:])
                    nc.sync.dma_start(out=dbg["QT"][:], in_=dbg_qt[:])
                    nc.sync.dma_start(out=dbg["KT"][:], in_=dbg_kt[:])
                    nc.sync.dma_start(out=dbg["V"][:],
                                      in_=state[("V", 0)][:69])
                    nc.sync.dma_start(out=dbg["rr0"][:],
                                      in_=state[("rr2b", 0)][64:66])
                    nc.sync.dma_start(out=dbg["OTraw0"][:],
                                      in_=state[("OT", 0)][:])
                    nc.sync.dma_start(out=dbg["OTn0"][:],
                                      in_=state[("OTn", 0, 0)][:])
    nc.compile()
    return nc


_NC_CACHE = {}


def _get_nc(B_L):
    if B_L not in _NC_CACHE:
        _NC_CACHE[B_L] = build_nc(B_L)
    return _NC_CACHE[B_L]


def _make_in_maps(inputs, B_L):
    """Per-core input maps; x and W are host-cast to bf16 (the kernel
    consumed them in bf16 anyway -- this halves the input DMA)."""
    import ml_dtypes
    bf16 = ml_dtypes.bfloat16
    shared = {k: (np.ascontiguousarray(inputs[k].astype(bf16))
                  if k.startswith("W") else inputs[k])
              for k in ("Wq", "bq", "Wk", "bk", "Wv", "bv", "Wp", "bp")}
    xb = {nm: inputs[nm].astype(bf16) for nm in ("x_base", "x_target")}
    in_maps = []
    for i in range(N_CORES):
        m = dict(shared)
        m["x_base"] = np.ascontiguousarray(xb["x_base"][i * B_L:(i + 1) * B_L])
        m["x_target"] = np.ascontiguousarray(
            xb["x_target"][i * B_L:(i + 1) * B_L])
        in_maps.append(m)
    return in_maps


def kernel(**inputs):
    inputs = {k: np.ascontiguousarray(np.asarray(v), dtype=np.float32)
              for k, v in inputs.items()}
    B = inputs["x_base"].shape[0]
    assert B % N_CORES == 0, f"batch {B} not divisible by {N_CORES} cores"
    B_L = B // N_CORES
    nc = _get_nc(B_L)
    in_maps = _make_in_maps(inputs, B_L)
    res = run_bass_kernel_spmd(nc, in_maps, core_ids=list(range(N_CORES)))
    return np.concatenate([r["out"] for r in res.results], axis=1)


# revision 39
# speedup vs baseline: 1.1755x; 1.1755x over previous
"""Trainium2 Bass kernel for nn_Attention_86217173500445.

Cross-attention block: shared QKV projections over two inputs (base/target),
4 attention streams (bb, tt, bt, tb), shared output projection.

Strategy: data-parallel over batch (B=32 -> 4 per core on 8 cores), weights
replicated, zero collectives.  Per-core compute is a fully-fused bf16
pipeline (1 column/cycle on the PE, fp32 PSUM accumulation; rel err ~7e-3
vs the 2e-2 gate):

  - x and the 4 weight matrices are HOST-cast to bf16 and uploaded as bf16
    (the kernel consumed them in bf16 anyway) -- halves the input DMA and
    removes all on-chip staging casts.
  - x is transposed on-chip at the bf16 1-cycle/row rate into XT [C, S].
  - Q/K projections produce transposed outputs QT/KT [C, S] directly
    (bias applied by the ACT psum drain); V projection produces
    natural-layout V [S, C] (bias added by the DVE drain).
  - Scores are computed transposed (scoresT[k, q]) so the ACT-engine exp
    output feeds the AV matmul as the moving operand with no transposes.
    Max-subtraction is skipped (scores ~ N(0,1), exp is safe).
  - V carries two trailing all-ones columns, so each AV matmul lands the
    head's softmax row-sum in psum rows 64/65 for free (the dedicated
    row-sum matmuls of the earlier version were ~60us of PE time).
  - Normalization: the two rowsum rows are copied (ACT j1 / DVE j0) to a
    [66, ...] SBUF tile, broadcast along channels by a 2-row stationary
    matmul at tile position (64, 0), reciprocal'd on DVE *after* the
    broadcast (reciprocal_approx_fast is FD-bound, so 128 rows cost the
    same as 2 -- and the custom uop misreads psum at a nonzero base
    partition on HW, so it must run at base 0), and applied by a GpSimd
    multiply (DVE multiply for the last batch, where GpSimd serialization
    would stretch the epilogue).
  - Output projection consumes the normalized attention output as the
    stationary operand, producing natural [S, C] tiles DMA'd to DRAM in
    per-512/256-channel chunks alternating between two DMA queues.

Scheduling: engines execute their queues strictly in order, so the static
emission order IS the schedule.  The PE clock is HAM-gated: it runs at
1.2 GHz unless the PE stays busy through consecutive 3.4us activity
windows, so every pair slot must carry enough independent PE work:
  - a dense dummy-matmul burst under the (DMA-bound) weight-load prologue
    warms the PE clock before real work,
  - per pair slot: scores(sti0) / AV(j0, pair-3) / scores(sti1) /
    AV(j1, pair-3) / rowsum-broadcast(pair-5) are interleaved so the PE
    never catches up with the ACT exp chain,
  - PE fillers: prev batch's sigma-1 outproj at slots 0-2/4, next batch's
    transposes at 3/5, V projection at 6-7, Q/K projection at 8-9 (spilling
    into the tail), this batch's sigma-0 outproj at 10-11; slots with no
    real fill get dummy-matmul padding so the HAM window never reads idle.
Engine balance: exp + QK-bias drains + rowsum j1 copies on ACT; OT drains
(copy + partition-shift shuffle), rowsum j0 copies, reciprocals and V/out
biases on DVE; normalize multiplies and out-DMA issue on GpSimd.
"""

import numpy as np

import concourse.bass as bass
import concourse.bacc as bacc
import concourse.mybir as mybir
import concourse.tile as tile
from concourse.bass_utils import run_bass_kernel_spmd
from concourse.masks import make_identity

FP32 = mybir.dt.float32
BF16 = mybir.dt.bfloat16
AF = mybir.ActivationFunctionType

H, DH, S, C = 12, 64, 197, 768
NCO = C // 128  # 6 channel chunks
SCALE = DH ** -0.5
S_TILES = [(0, 128), (128, 69)]
N_CHUNKS = [(0, 512), (512, 256)]
# (key/value source, query source) -> output stream index; 0=base, 1=target
STREAM_IDX = {(0, 0): 0, (0, 1): 3, (1, 1): 1, (1, 0): 2}
N_CORES = 8
S2 = 2 * S  # query axis covers both query sources side by side
DV = DH + 2  # V head stride: 64 data columns + 2 all-ones (rowsum) columns
DEBUG_DUMPS = False


def build_nc(B_L):
    nc = bacc.Bacc("TRN2", target_bir_lowering=False, debug=False,
                   num_devices=N_CORES)

    x_in = {
        0: nc.dram_tensor("x_base", [B_L, S, C], BF16, kind="ExternalInput"),
        1: nc.dram_tensor("x_target", [B_L, S, C], BF16, kind="ExternalInput"),
    }
    w_dram, b_dram = {}, {}
    for nm in ("q", "k", "v", "p"):
        w_dram[nm] = nc.dram_tensor(f"W{nm}", [C, C], BF16, kind="ExternalInput")
        b_dram[nm] = nc.dram_tensor(f"b{nm}", [C], FP32, kind="ExternalInput")
    out_d = nc.dram_tensor("out", [4, B_L, S, C], FP32, kind="ExternalOutput")
    dbg = {}
    if DEBUG_DUMPS:
        dbg["XT"] = nc.dram_tensor("dbg_XT", [128, NCO, 2, S], BF16,
                                   kind="ExternalOutput")
        dbg["QT"] = nc.dram_tensor("dbg_QT", [128, NCO, 2, S], BF16,
                                   kind="ExternalOutput")
        dbg["KT"] = nc.dram_tensor("dbg_KT", [128, NCO, 2, S], BF16,
                                   kind="ExternalOutput")
        dbg["V"] = nc.dram_tensor("dbg_V", [69, 2, 2, H, DV], BF16,
                                  kind="ExternalOutput")
        dbg["rr0"] = nc.dram_tensor("dbg_rr0", [2, NCO, S2], BF16,
                                    kind="ExternalOutput")
        dbg["OTraw0"] = nc.dram_tensor("dbg_OTraw0", [128, NCO, S2], FP32,
                                       kind="ExternalOutput")
        dbg["OTn0"] = nc.dram_tensor("dbg_OTn0", [128, NCO, S2], BF16,
                                     kind="ExternalOutput")

    with tile.TileContext(nc) as tc:
        with (
            tc.tile_pool(name="const", bufs=1) as constp,
            tc.tile_pool(name="stage", bufs=4) as stagep,
            tc.tile_pool(name="wsb", bufs=1) as wp,
            tc.tile_pool(name="xt", bufs=2) as xtp,
            tc.tile_pool(name="qkv", bufs=2) as qkvp,
            tc.tile_pool(name="expp", bufs=20) as expp,
            tc.tile_pool(name="ot", bufs=2) as otp,
            tc.tile_pool(name="rpool", bufs=2) as rp,
            tc.tile_pool(name="y2", bufs=4) as y2p,
            tc.tile_pool(name="ps_sc", bufs=3, space="PSUM") as ps_sc,
            tc.tile_pool(name="ps_av", bufs=2, space="PSUM") as ps_av,
            tc.tile_pool(name="ps_sh", bufs=2, space="PSUM") as ps_sh,
            tc.tile_pool(name="ps_pr", bufs=1, space="PSUM") as ps_pr,
        ):
            # ---- constants ----
            ident = constp.tile([128, 128], BF16)
            make_identity(nc, ident)

            # E2[64, c] = 1 iff c < 64; E2[65, c] = 1 iff c >= 64.  The
            # 2-row stationary that broadcasts the per-head (j0, j1)
            # 1/rowsum rows across their 64-channel groups.
            E2 = constp.tile([66, 128], BF16, name="E2")
            nc.gpsimd.memset(E2, 1.0)
            nc.gpsimd.affine_select(
                out=E2[64:66, :], in_=E2[64:66, :],
                compare_op=mybir.AluOpType.is_ge, fill=0.0,
                base=0, pattern=[[1, 128]], channel_multiplier=-DH)
            nc.gpsimd.affine_select(
                out=E2[64:66, :], in_=E2[64:66, :],
                compare_op=mybir.AluOpType.is_ge, fill=0.0,
                base=DH - 1, pattern=[[-1, 128]], channel_multiplier=DH)

            # per-partition channel biases for the transposed Q/K outputs
            bqk_sb = {}
            for nm in ("q", "k"):
                t = constp.tile([128, NCO], FP32, name=f"b{nm}_sb")
                nc.gpsimd.dma_start(
                    out=t, in_=b_dram[nm].rearrange("(ko p) -> p ko", p=128))
                bqk_sb[nm] = t
            # V / out-proj biases broadcast along partitions (DVE add)
            bbc = {}
            for nm in ("v", "p"):
                t = constp.tile([128, C], FP32, name=f"b{nm}_bc")
                src_ap = b_dram[nm][:]
                bcast = bass.AP(tensor=src_ap.tensor, offset=src_ap.offset,
                                ap=[[0, 128]] + list(src_ap.ap))
                nc.gpsimd.dma_start(out=t, in_=bcast)
                bbc[nm] = t
            bbc_v, bbc_p = bbc["v"], bbc["p"]

            # ---- PE warm-up: dense dummy matmuls under the weight-load
            # prologue so HAM un-throttles the PE clock before real work ----
            warm_w = constp.tile([128, 512], BF16, name="warm_w")
            nc.vector.memset(warm_w, 0.125)

            def emit_warm(n):
                for _ in range(n):
                    wp_ = ps_sc.tile([128, 512], FP32, tag="sc", name="warm_ps")
                    nc.tensor.matmul(wp_[:, :512], lhsT=warm_w[:, :128],
                                     rhs=warm_w[:, :512], start=True, stop=True)

            emit_warm(48)

            # ---- prefetch batch-0 x tiles ahead of the weight loads ----
            x_tiles = {}

            def emit_x_dma(b):
                for src in (0, 1):
                    for (s0, s_sz) in S_TILES:
                        xb = stagep.tile([128, C], BF16, tag="xb", name="xb")
                        nc.sync.dma_start(out=xb[:s_sz, :],
                                          in_=x_in[src][b, s0:s0 + s_sz, :])
                        x_tiles[(b, src, s0)] = xb

            emit_x_dma(0)

            # ---- weights: DMA fp32 then DVE-cast to bf16 ----
            W_sb = {}

            def emit_w_load(nm):
                W_sb[nm] = wp.tile([128, NCO, C], BF16, tag=f"w{nm}",
                                   name=f"W{nm}_sb")
                for ko in range(NCO):
                    nc.sync.dma_start(out=W_sb[nm][:, ko, :],
                                      in_=w_dram[nm][ko * 128:(ko + 1) * 128, :])

            for nm in ("v", "q", "k"):
                emit_w_load(nm)

            # ---- per-batch persistent tiles, (re)allocated each iteration ----
            state = {}

            def emit_transpose_piece(b, src, sti, use_act):
                """Transpose one (src, s-tile) slab of x into XT: 6 channel
                chunks as two psum-bank groups, each drained by one bulk
                copy so the phase stays PE-dense instead of copy-paced."""
                s0, s_sz = S_TILES[sti]
                xb = x_tiles[(b, src, s0)]
                XT = state[("XT", b)]
                for g, (c0, ncg) in enumerate(((0, 4), (4, 2))):
                    pt = ps_sh.tile([128, 4, 128], BF16, tag="sh",
                                    name="pt")
                    for ci in range(ncg):
                        co = c0 + ci
                        nc.tensor.transpose(
                            pt[:, ci, :s_sz],
                            xb[:s_sz, co * 128:(co + 1) * 128],
                            ident[:s_sz, :s_sz])
                    dst = XT[:, c0:c0 + ncg, src, s0:s0 + s_sz]
                    if use_act and (src + g) % 2 == 0:
                        nc.scalar.copy(out=dst, in_=pt[:, :ncg, :s_sz])
                    else:
                        nc.vector.tensor_copy(out=dst, in_=pt[:, :ncg, :s_sz])

            def emit_transposes(b, pieces=None, use_act=True):
                if ("XT", b) not in state:
                    state[("XT", b)] = xtp.tile([128, NCO, 2, S], BF16,
                                                tag="xt", name="XT")
                if pieces is None:
                    pieces = [(src, sti) for src in (0, 1) for sti in (0, 1)]
                for src, sti in pieces:
                    emit_transpose_piece(b, src, sti, use_act=use_act)

            def _emit_qk_one(nm, OUT, m, b):  # noqa: unused b kept
                XT = state[("XT", b)]
                pp = ps_sh.tile([128, 2, S], FP32, tag="sh", name="pp")
                for k in range(NCO):
                    nc.tensor.matmul(
                        pp[:], lhsT=W_sb[nm][:, k, m * 128:(m + 1) * 128],
                        rhs=XT[:, k, :, :],
                        start=(k == 0), stop=(k == NCO - 1))
                nc.scalar.activation(
                    out=OUT[:, m, :, :], in_=pp[:], func=AF.Identity,
                    bias=bqk_sb[nm][:, m:m + 1], scale=1.0)

            def emit_qk_half(b, half):
                """Q/K projection chunks m in [3*half, 3*half+3)."""
                if half == 0:
                    state[("QT", b)] = qkvp.tile([128, NCO, 2, S], BF16,
                                                 tag="qt", name="QT")
                    state[("KT", b)] = qkvp.tile([128, NCO, 2, S], BF16,
                                                 tag="kt", name="KT")
                for m in range(3 * half, 3 * half + 3):
                    _emit_qk_one("q", state[("QT", b)], m, b)
                for m in range(3 * half, 3 * half + 3):
                    _emit_qk_one("k", state[("KT", b)], m, b)

            def emit_vproj_half(b, src):
                """V projection for one source; bias rides the matmul as a
                ones-row accumulation, so the drain is a plain DVE copy."""
                XT = state[("XT", b)]
                if src == 0:
                    V_sb = qkvp.tile([128, 2, 2, H, DV], BF16, tag="v",
                                     name="V_sb")
                    state[("V", b)] = V_sb
                    # the two all-ones rowsum columns per head
                    nc.gpsimd.memset(V_sb[:, :, :, :, DH:DV], 1.0)
                V_sb = state[("V", b)]
                for sti, (s0, s_sz) in enumerate(S_TILES):
                    for (n0, n_sz) in N_CHUNKS:
                        pv = ps_sh.tile([128, 512], FP32, tag="sh",
                                        name="pv")
                        for k in range(NCO):
                            nc.tensor.matmul(
                                pv[:s_sz, :n_sz],
                                lhsT=XT[:, k, src, s0:s0 + s_sz],
                                rhs=W_sb["v"][:, k, n0:n0 + n_sz],
                                start=(k == 0), stop=(k == NCO - 1))
                        nh, h0 = n_sz // DH, n0 // DH
                        nc.vector.tensor_add(
                            out=V_sb[:s_sz, src, sti, h0:h0 + nh, :DH],
                            in0=pv[:s_sz, :n_sz].rearrange(
                                "p (h d) -> p h d", d=DH),
                            in1=bbc_v[:s_sz, n0:n0 + n_sz].rearrange(
                                "p (h d) -> p h d", d=DH))

            def emit_proj(b):
                emit_transposes(b, use_act=False)
                emit_vproj_half(b, 0)
                emit_qk_half(b, 0)
                emit_vproj_half(b, 1)
                emit_qk_half(b, 1)

            def emit_scores_exp(b, sigma, hh, sti):
                """Scores + exp for head pair hh, one s-tile."""
                QT, KT = state[("QT", b)], state[("KT", b)]
                s0, s_sz = S_TILES[sti]
                et = state.setdefault(("e", sigma, hh), {})
                for j in (0, 1):
                    hp = j * DH
                    psc = ps_sc.tile([128, 512], FP32, tag="sc", name="psc")
                    nc.tensor.matmul(
                        psc[:s_sz, :S2],
                        lhsT=KT[hp:hp + DH, hh, sigma, s0:s0 + s_sz],
                        rhs=QT[hp:hp + DH, hh, :, :],
                        start=True, stop=True)
                    e = expp.tile([128, S2], BF16, tag="exp", name="e")
                    nc.scalar.activation(out=e[:s_sz, :],
                                         in_=psc[:s_sz, :S2],
                                         func=AF.Exp, scale=float(SCALE))
                    et[(sti, j)] = e

            def emit_av_mms(b, sigma, hh, j):
                """AV matmuls for one head of pair hh (rowsum rides rows
                64/65 via the ones columns of V)."""
                V_sb = state[("V", b)]
                et = state[("e", sigma, hh)]
                h = 2 * hh + j
                pav = ps_av.tile([128, 512], FP32, tag="av", name="pav")
                for sti, (s0, s_sz) in enumerate(S_TILES):
                    nc.tensor.matmul(
                        pav[:DV, :S2],
                        lhsT=V_sb[:s_sz, sigma, sti, h, :],
                        rhs=et[(sti, j)][:s_sz, :],
                        start=(sti == 0), stop=(sti == 1))
                state[("pav", sigma, hh, j)] = pav

            def emit_av_drains(sigma, hh):
                """Drain O rows to OT_raw, 1/rowsum to rr2b (bf16)."""
                OT_raw = state[("OT", sigma)]
                rr2b = state[("rr2b", sigma)]
                state.pop(("e", sigma, hh))
                pav0 = state.pop(("pav", sigma, hh, 0))
                pav1 = state.pop(("pav", sigma, hh, 1))
                nc.vector.tensor_copy(out=OT_raw[0:DH, hh, :],
                                       in_=pav0[0:DH, :S2])
                nc.vector.stream_shuffle(
                    out=OT_raw[DH:2 * DH, hh, :],
                    in_=pav1[0:DH, :S2], mask=list(range(32)))
                # rowsum rows stay at partitions 64/65 (bf16): j1's copy
                # fills both, j0's overwrites row 64
                nc.scalar.copy(out=rr2b[64:66, hh, :],
                               in_=pav1[64:66, :S2])
                nc.vector.tensor_copy(out=rr2b[64:65, hh, :],
                                      in_=pav0[64:65, :S2])

            def emit_normpair(b, sigma, hh):
                """Broadcast the rowsums along channels (PE), reciprocal of
                the broadcast (DVE, base 0), normalize multiply (GpSimd)."""
                OT_raw = state[("OT", sigma)]
                rr2b = state[("rr2b", sigma)]
                OT = state[("OTn", b, sigma)]
                pr = ps_pr.tile([128, 512], FP32, tag="pr", name="pr")
                nc.tensor.matmul(pr[:, :S2],
                                 lhsT=E2[64:66, :],
                                 rhs=rr2b[64:66, hh, :],
                                 start=True, stop=True)
                rbc = rp.tile([128, S2], FP32, tag="rbc", name="rbc", bufs=2)
                nc.vector.reciprocal_approx_fast(out=rbc, in_=pr[:, :S2])
                if b == B_L - 1 and (sigma, hh) >= (1, 2):
                    mul = nc.vector.tensor_mul if hh % 2 else \
                        nc.gpsimd.tensor_mul
                else:
                    mul = nc.gpsimd.tensor_mul
                mul(out=OT[:, hh, :], in0=OT_raw[:, hh, :], in1=rbc)

            def emit_outproj(b, sigma, qs, sti):
                """One [s_tile, C] slab of the output projection."""
                OT = state[("OTn", b, sigma)]
                stream = STREAM_IDX[(sigma, qs)]
                s0, s_sz = S_TILES[sti]
                y = y2p.tile([128, C], FP32, tag="y2")
                for (n0, n_sz) in N_CHUNKS:
                    py = ps_sh.tile([128, 512], FP32, tag="sh", name="py")
                    for k in range(NCO):
                        nc.tensor.matmul(
                            py[:s_sz, :n_sz],
                            lhsT=OT[:, k, qs * S + s0: qs * S + s0 + s_sz],
                            rhs=W_sb["p"][:, k, n0:n0 + n_sz],
                            start=(k == 0), stop=(k == NCO - 1))
                    nc.vector.tensor_add(
                        out=y[:s_sz, n0:n0 + n_sz],
                        in0=py[:s_sz, :n_sz],
                        in1=bbc_p[:s_sz, n0:n0 + n_sz])
                    dma_q = nc.sync if (qs + sti + (n0 > 0)) % 2 == 0 \
                        else nc.gpsimd
                    dma_q.dma_start(
                        out=out_d[stream, b, s0:s0 + s_sz, n0:n0 + n_sz],
                        in_=y[:s_sz, n0:n0 + n_sz])

            # ---- main loop: software-pipelined emission.  Tail work and
            # the next batch's projections are spread across the pair
            # slots so the PE always has independent fill work behind the
            # exp dependency chain. ----
            emit_proj(0)
            emit_w_load("p")
            for b in range(B_L):
                for sigma in (0, 1):
                    state[("OT", sigma)] = otp.tile(
                        [128, NCO, S2], FP32, tag="otraw", name="OT", bufs=2)
                    state[("rr2b", sigma)] = rp.tile(
                        [66, NCO, S2], BF16, tag="rr2b", name="rr2b", bufs=2)
                    state[("OTn", b, sigma)] = otp.tile(
                        [128, NCO, S2], BF16, tag="ot", name="OTn", bufs=3)
                pairs = [(sigma, hh) for sigma in (0, 1) for hh in range(NCO)]
                if DEBUG_DUMPS and b == 0:
                    dbg_qt, dbg_kt = state[("QT", 0)], state[("KT", 0)]
                if b + 1 < B_L:
                    emit_x_dma(b + 1)
                for idx, (sigma, hh) in enumerate(pairs):
                    # interleave scores with the AV matmuls of pair idx-3
                    # (3 slots of lead so the PE never catches up with the
                    # ACT exp chain) and the rowsum broadcast of pair idx-5
                    emit_scores_exp(b, sigma, hh, 0)
                    if idx > 2:
                        emit_av_mms(b, *pairs[idx - 3], 0)
                    emit_scores_exp(b, sigma, hh, 1)
                    if idx > 2:
                        emit_av_mms(b, *pairs[idx - 3], 1)
                        emit_av_drains(*pairs[idx - 3])
                    if idx > 4:
                        emit_normpair(b, *pairs[idx - 5])
                    # PE fill: every slot gets independent matmul work (an
                    # idle PE gets clocked down to 1.2 GHz): prev batch's
                    # sigma-1 outproj at 0-2/4, next batch's transposes at
                    # 3/5 and V projection at 6-7, Q/K projection at 8-9,
                    # this batch's first sigma-0 outproj at 10-11.  Slots
                    # with no fill available get dummy-matmul padding.
                    fill = False
                    if idx in (0, 1, 2, 4):
                        if b > 0:
                            sl = idx if idx < 3 else 3
                            emit_outproj(b - 1, 1, sl // 2, sl % 2)
                            fill = True
                    elif idx in (3, 5):
                        if b + 1 < B_L:
                            s = 0 if idx == 3 else 1
                            emit_transposes(b + 1, [(s, 0), (s, 1)])
                            fill = True
                    elif idx in (6, 7):
                        if b + 1 < B_L:
                            emit_vproj_half(b + 1, idx - 6)
                            fill = True
                    elif idx == 8:
                        if b + 1 < B_L:
                            emit_qk_half(b + 1, 0)
                            fill = True
                    elif idx == 9:
                        fill = b + 1 < B_L  # qk_half(0) spills into this slot
                    elif idx >= 10:
                        emit_outproj(b, 0, (idx - 10) // 2, (idx - 10) % 2)
                        fill = True
                    if not fill:
                        emit_warm(9 if b == 0 else 3)
                # drain the last three pairs, interleaved with the next
                # batch's remaining projections so the XT-drain -> QK
                # dependency and the exp -> AV chains never idle the PE
                emit_av_mms(b, *pairs[9], 0)
                emit_av_mms(b, *pairs[9], 1)
                emit_av_drains(*pairs[9])
                if b + 1 < B_L:
                    emit_qk_half(b + 1, 1)
                else:
                    emit_warm(2)
                emit_av_mms(b, *pairs[10], 0)
                emit_normpair(b, *pairs[7])
                emit_av_mms(b, *pairs[10], 1)
                emit_av_drains(*pairs[10])
                emit_normpair(b, *pairs[8])
                emit_outproj(b, 0, 1, 0)
                emit_av_mms(b, *pairs[11], 0)
                emit_normpair(b, *pairs[9])
                emit_av_mms(b, *pairs[11], 1)
                emit_av_drains(*pairs[11])
                emit_normpair(b, *pairs[10])
                emit_outproj(b, 0, 1, 1)
                emit_normpair(b, *pairs[11])
                if b + 1 >= B_L:
                    emit_warm(6)
                    emit_outproj(b, 1, 0, 0)
                    emit_warm(3)
                    emit_outproj(b, 1, 0, 1)
                    emit_warm(3)
                    emit_outproj(b, 1, 1, 0)
                    emit_outproj(b, 1, 1, 1)
                if DEBUG_DUMPS and b == 0:
                    nc.sync.dma_start(out=dbg["XT"][:], in_=state[("XT", 0)][:])
                    nc.sync.dma_start(out=dbg["QT"][:], in_=dbg_qt[:])
                    nc.sync.dma_start(out=dbg["KT"][:], in_=dbg_kt[:])
                    nc.sync.dma_start(out=dbg["V"][:],
                                      in_=state[("V", 0)][:69])
                    nc.sync.dma_start(out=dbg["rr0"][:],
                                      in_=state[("rr2b", 0)][64:66])
                    nc.sync.dma_start(out=dbg["OTraw0"][:],
                                      in_=state[("OT", 0)][:])
                    nc.sync.dma_start(out=dbg["OTn0"][:],
                                      in_=state[("OTn", 0, 0)][:])
    nc.compile()
    return nc


_NC_CACHE = {}


def _get_nc(B_L):
    if B_L not in _NC_CACHE:
        _NC_CACHE[B_L] = build_nc(B_L)
    return _NC_CACHE[B_L]


def _make_in_maps(inputs, B_L):
    """Per-core input maps; x and W are host-cast to bf16 (the kernel
    consumed them in bf16 anyway -- this halves the input DMA)."""
    import ml_dtypes
    bf16 = ml_dtypes.bfloat16
    shared = {k: (np.ascontiguousarray(inputs[k].astype(bf16))
                  if k.startswith("W") else inputs[k])
              for k in ("Wq", "bq", "Wk", "bk", "Wv", "bv", "Wp", "bp")}
    xb = {nm: inputs[nm].astype(bf16) for nm in ("x_base", "x_target")}
    in_maps = []
    for i in range(N_CORES):
        m = dict(shared)
        m["x_base"] = np.ascontiguousarray(xb["x_base"][i * B_L:(i + 1) * B_L])
        m["x_target"] = np.ascontiguousarray(
            xb["x_target"][i * B_L:(i + 1) * B_L])
        in_maps.append(m)
    return in_maps


def kernel(**inputs):
    inputs = {k: np.ascontiguousarray(np.asarray(v), dtype=np.float32)
              for k, v in inputs.items()}
    B = inputs["x_base"].shape[0]
    assert B % N_CORES == 0, f"batch {B} not divisible by {N_CORES} cores"
    B_L = B // N_CORES
    nc = _get_nc(B_L)
    in_maps = _make_in_maps(inputs, B_L)
    res = run_bass_kernel_spmd(nc, in_maps, core_ids=list(range(N_CORES)))
    return np.concatenate([r["out"] for r in res.results], axis=1)
